# revision 6
# baseline (speedup 1.0000x reference)
"""Bahdanau attention Trainium2 kernel.

Computes, for hidden [B,H], encoder_outputs [B,S,H], W_a [H,H], U_a [H,H], v_a [H]:
    wh      = hidden @ W_a.T                      [B,H]
    ue      = enc @ U_a.T                         [B,S,H]
    score   = tanh(wh[:,None,:] + ue) . v_a       [B,S]
    attn    = softmax(score, axis=1)              [B,S]
    context = attn . enc                          [B,H]
returns (context, attn).

Sharding: data-parallel over batch, B=32 -> 4 per core across 8 cores.

Per-core plan (B_L=4, S=2048, H=1024, R=B_L*S rows):
  - ue computed transposed: psum[o_chunk(128), r_block(512)] accumulating over
    8 h-chunks; lhsT = U_a^T tiles (SBUF-resident), rhs = X^T tiles where
    X^T[h, r] = enc_flat[r, h] is pre-transposed on host.  fp32r matmuls.
  - ACT fuses tanh(ue + wh) using per-partition bias = wh^T[:, b] column.
  - score = v . tanh via PE: lhsT = v chunk [128,1], rhs = tanh tile [128,512],
    accumulated over o-chunks into psum [1, 512].
  - softmax over S without max-subtraction (|score| <= sum|v_a| ~ 16, exp safe
    in fp32); exp on ACT with accum_out partial sums; reciprocal on DVE.
  - attn row round-trips through a DRAM scratch to get attn^T [128,16] per b.
  - context[b] = attn^T-cols @ enc natural-layout tiles [128 s, 512 h],
    accumulated over 16 s-chunks into psum [1, 512] per h-half.
"""

import sys

sys.path.insert(0, "/opt/trn_rl_repo")

import numpy as np

import concourse.bass as bass
import concourse.tile as tile
from concourse import bacc, mybir
from concourse import bass_utils

F32 = mybir.dt.float32
F32R = mybir.dt.float32r
BF16 = mybir.dt.bfloat16

N_CORES = 8
B = 32
S_FULL = 2048
H = 1024
B_L = B // N_CORES  # 4 batches per core

HC = H // 128  # 8 h-chunks
OC = H // 128  # 8 o-chunks


def build_program(reps: int = 1, mode: str = "f32r", s: int = S_FULL):
    """Build the per-core Bass program.

    reps > 1 wraps the whole body in a tc.For_i loop for wall-clock timing.
    mode: "f32r" (fp32 data, float32r matmuls), "bf16" (bf16 data/matmuls),
          "f32"  (plain fp32 matmuls, 4x slower - baseline only).
    """
    assert s % 512 == 0 and (s // 512) >= 1
    R = B_L * s
    NBLK = R // 512  # r-blocks of 512 rows
    BPB = s // 512  # r-blocks per batch element

    in_dt = {"f32r": F32R, "bf16": BF16, "f32": F32}[mode]

    def mmcast(ap):
        return ap

    nc = bacc.Bacc("TRN2", target_bir_lowering=False, debug=False,
                   num_devices=N_CORES)

    xT = nc.dram_tensor("xT", [H, R], in_dt, kind="ExternalInput").ap()
    enc = nc.dram_tensor("enc", [B_L, s, H], in_dt, kind="ExternalInput").ap()
    uaT = nc.dram_tensor("uaT", [H, H], in_dt, kind="ExternalInput").ap()
    waT = nc.dram_tensor("waT", [H, H], in_dt, kind="ExternalInput").ap()
    hidT = nc.dram_tensor("hidT", [H, B_L], in_dt, kind="ExternalInput").ap()
    vv = nc.dram_tensor("vv", [128, OC], in_dt, kind="ExternalInput").ap()
    ctx_out = nc.dram_tensor("ctx_out", [B_L, H], F32, kind="ExternalOutput").ap()
    attn_out = nc.dram_tensor("attn_out", [B_L, s], F32, kind="ExternalOutput").ap()

    with tile.TileContext(nc) as tc:
        with (
            tc.tile_pool(name="consts", bufs=1) as consts,
            tc.tile_pool(name="xt", bufs=2) as xt_p,
            tc.tile_pool(name="tanh", bufs=3) as tanh_p,
            tc.tile_pool(name="softmax", bufs=2) as sm_p,
            tc.tile_pool(name="encp", bufs=2) as enc_p,
            tc.tile_pool(name="attnT", bufs=2) as attnT_p,
            tc.tile_pool(name="small", bufs=4) as small_p,
            tc.tile_pool(name="uep", bufs=3, space="PSUM") as ue_ps_p,
            tc.tile_pool(name="scorep", bufs=2, space="PSUM") as score_ps_p,
            tc.tile_pool(name="ctxp", bufs=2, space="PSUM") as ctx_ps_p,
            tc.tile_pool(name="dram", bufs=2, space="DRAM") as dram_p,
        ):
            def body(_i=None):
                # ---- constants ----
                uaT_sb = consts.tile([128, HC, H], in_dt, tag="uaT")
                nc.sync.dma_start(
                    uaT_sb[:], uaT.rearrange("(c p) o -> p c o", p=128))
                waT_sb = consts.tile([128, HC, H], in_dt, tag="waT")
                nc.sync.dma_start(
                    waT_sb[:], waT.rearrange("(c p) o -> p c o", p=128))
                hidT_sb = consts.tile([128, HC, B_L], in_dt, tag="hidT")
                nc.sync.dma_start(
                    hidT_sb[:], hidT.rearrange("(c p) b -> p c b", p=128))
                v_sb = consts.tile([128, OC], in_dt, tag="v")
                nc.sync.dma_start(v_sb[:], vv[:])
                whT_sb = consts.tile([128, OC, B_L], F32, tag="whT")

                # ---- wh^T = W_a @ hidden^T  -> [o, b] per o-chunk ----
                for oc in range(OC):
                    whT_ps = ue_ps_p.tile([128, B_L], F32, tag="ue")
                    for hc in range(HC):
                        nc.tensor.matmul(
                            whT_ps[:],
                            mmcast(waT_sb[:, hc, oc * 128:(oc + 1) * 128]),
                            mmcast(hidT_sb[:, hc, :]),
                            start=(hc == 0), stop=(hc == HC - 1),
                        )
                    nc.scalar.copy(whT_sb[:, oc, :], whT_ps[:])

                # state carried per batch element
                exp_sb = {}
                sums_sb = {}
                inv_sb = {}

                def emit_block(rb):
                    b = rb // BPB
                    xt_t = xt_p.tile([128, HC, 512], in_dt, tag="xt")
                    nc.sync.dma_start(
                        xt_t[:],
                        xT.rearrange("(c p) r -> p c r", p=128)[
                            :, :, rb * 512:(rb + 1) * 512],
                    )
                    score_ps = score_ps_p.tile([1, 512], F32, tag="score")
                    pend = []  # delayed v-dot matmuls to keep PE dense
                    for oc in range(OC):
                        ue_ps = ue_ps_p.tile([128, 512], F32, tag="ue")
                        for hc in range(HC):
                            nc.tensor.matmul(
                                ue_ps[:],
                                mmcast(uaT_sb[:, hc, oc * 128:(oc + 1) * 128]),
                                mmcast(xt_t[:, hc, :]),
                                start=(hc == 0), stop=(hc == HC - 1),
                            )
                        tanh_t = tanh_p.tile([128, 512], in_dt, tag="tanh")
                        nc.scalar.activation(
                            tanh_t[:], ue_ps[:],
                            mybir.ActivationFunctionType.Tanh,
                            bias=whT_sb[:, oc, b:b + 1],
                        )
                        if pend:
                            pend.pop()()
                        pend.append(lambda oc=oc, t=tanh_t: nc.tensor.matmul(
                            score_ps[:],
                            mmcast(v_sb[:, oc:oc + 1]),
                            mmcast(t[:]),
                            start=(oc == 0), stop=(oc == OC - 1),
                        ))
                    pend.pop()()

                    # exp of this block's scores, with partial sum
                    j = rb % BPB
                    if j == 0:
                        exp_sb[b] = sm_p.tile([1, s], F32, tag="exp", name=f"exp{b}")
                        sums_sb[b] = small_p.tile([1, BPB], F32, tag="sums", name=f"sums{b}")
                    nc.scalar.activation(
                        exp_sb[b][:, j * 512:(j + 1) * 512], score_ps[:],
                        mybir.ActivationFunctionType.Exp,
                        accum_out=sums_sb[b][:, j:j + 1],
                    )

                def emit_softmax(b):
                    tot = small_p.tile([1, 1], F32, tag="tot")
                    nc.vector.tensor_reduce(
                        tot[:], sums_sb[b][:], axis=mybir.AxisListType.X,
                        op=mybir.AluOpType.add)
                    inv_sb[b] = small_p.tile([1, 1], F32, tag="inv", name=f"inv{b}")
                    nc.vector.reciprocal(inv_sb[b][:], tot[:])
                    attn_row = exp_sb[b]  # normalize in place
                    nc.vector.tensor_scalar_mul(
                        attn_row[:], exp_sb[b][:], inv_sb[b][:])
                    nc.sync.dma_start(attn_out[b:b + 1, :], attn_row[:])
                    # matmul-typed copy of the row for the transpose round-trip
                    attn_row_mm = sm_p.tile([1, s], in_dt, tag="attnmm",
                                            name=f"attnmm{b}")
                    nc.vector.tensor_copy(attn_row_mm[:], attn_row[:])
                    scr = dram_p.tile([1, s], in_dt, tag="scr")
                    nc.sync.dma_start(scr[:], attn_row_mm[:])
                    attnT = attnT_p.tile([128, s // 128], in_dt, tag="attnT")
                    nc.sync.dma_start(
                        attnT[:], scr[0].rearrange("(c p) -> p c", p=128))
                    return attnT

                def emit_context(b, attnT):
                    nsc = s // 128  # s-chunks
                    ngrp = max(1, nsc // 8)  # DMA groups of 8 s-chunks
                    gsz = nsc // ngrp
                    ctx_sb = small_p.tile([1, H], F32, tag="ctx")
                    for half in range(2):
                        ctx_ps = ctx_ps_p.tile([1, 512], F32, tag="ctx")
                        for g in range(ngrp):
                            enc_t = enc_p.tile([128, gsz, 512], in_dt, tag="enc")
                            nc.sync.dma_start(
                                enc_t[:],
                                enc[b].rearrange("(c p) (n h) -> p c n h",
                                                 p=128, n=2)[
                                    :, g * gsz:(g + 1) * gsz, half, :],
                            )
                            for k in range(gsz):
                                sc = g * gsz + k
                                nc.tensor.matmul(
                                    ctx_ps[:],
                                    mmcast(attnT[:, sc:sc + 1]),
                                    mmcast(enc_t[:, k, :]),
                                    start=(sc == 0), stop=(sc == nsc - 1),
                                )
                        nc.scalar.copy(
                            ctx_sb[:, half * 512:(half + 1) * 512], ctx_ps[:])
                    nc.sync.dma_start(ctx_out[b:b + 1, :], ctx_sb[:])

                attnT_ready = {}
                for rb in range(NBLK):
                    emit_block(rb)
                    b = rb // BPB
                    if rb % BPB == BPB - 1:
                        attnT_ready[b] = emit_softmax(b)
                    # emit context for previous b one block into the next b,
                    # so PE has ue work while softmax b completes
                    if rb % BPB == min(1, BPB - 1) and b > 0:
                        emit_context(b - 1, attnT_ready[b - 1])
                emit_context(B_L - 1, attnT_ready[B_L - 1])

            if reps == 1:
                body()
            else:
                with tc.For_i(0, reps, 1) as i:
                    body(i)

    nc.compile()
    return nc


def shard_inputs(hidden, encoder_outputs, W_a, U_a, v_a, mode="f32r",
                 s=S_FULL):
    """Build the 8 per-core input maps (numpy only; layout prep)."""
    np_dt = np.float32 if mode != "bf16" else None
    if mode == "bf16":
        import ml_dtypes
        np_dt = ml_dtypes.bfloat16

    def cast(x):
        return np.ascontiguousarray(x, dtype=np_dt)

    uaT = cast(np.asarray(U_a, np.float32).T)
    waT = cast(np.asarray(W_a, np.float32).T)
    vv = cast(np.asarray(v_a, np.float32).reshape(OC, 128).T)
    in_maps = []
    for c in range(N_CORES):
        bl = slice(c * B_L, (c + 1) * B_L)
        enc_l = np.asarray(encoder_outputs[bl], np.float32)
        hid_l = np.asarray(hidden[bl], np.float32)
        in_maps.append({
            "xT": cast(enc_l.reshape(B_L * s, H).T),
            "enc": cast(enc_l),
            "uaT": uaT,
            "waT": waT,
            "hidT": cast(hid_l.T),
            "vv": vv,
        })
    return in_maps


_prog_cache = {}


def _get_prog(reps=1, mode="f32r", s=S_FULL):
    key = (reps, mode, s)
    if key not in _prog_cache:
        _prog_cache[key] = build_program(reps, mode, s)
    return _prog_cache[key]


def run(hidden, encoder_outputs, W_a, U_a, v_a, mode="f32r", s=S_FULL,
        reps=1):
    nc = _get_prog(reps, mode, s)
    in_maps = shard_inputs(hidden, encoder_outputs, W_a, U_a, v_a, mode, s)
    res = bass_utils.run_bass_kernel_spmd(nc, in_maps,
                                          core_ids=list(range(N_CORES)))
    context = np.concatenate([res.results[c]["ctx_out"] for c in range(N_CORES)])
    attn = np.concatenate([res.results[c]["attn_out"] for c in range(N_CORES)])
    return context, attn


def kernel(hidden, encoder_outputs, W_a, U_a, v_a):
    return run(np.asarray(hidden), np.asarray(encoder_outputs),
               np.asarray(W_a), np.asarray(U_a), np.asarray(v_a))


# revision 7
# speedup vs baseline: 6.3040x; 6.3040x over previous
"""Bahdanau attention Trainium2 kernel.

Computes, for hidden [B,H], encoder_outputs [B,S,H], W_a [H,H], U_a [H,H], v_a [H]:
    wh      = hidden @ W_a.T                      [B,H]
    ue      = enc @ U_a.T                         [B,S,H]
    score   = tanh(wh[:,None,:] + ue) . v_a       [B,S]
    attn    = softmax(score, axis=1)              [B,S]
    context = attn . enc                          [B,H]
returns (context, attn).

Sharding: data-parallel over batch, B=32 -> 4 per core across 8 cores.

Per-core plan (B_L=4, S=2048, H=1024, R=B_L*S rows):
  - ue computed transposed: psum[o_chunk(128), r_block(512)] accumulating over
    8 h-chunks; lhsT = U_a^T tiles (SBUF-resident), rhs = X^T tiles where
    X^T[h, r] = enc_flat[r, h] is pre-transposed on host.  fp32r matmuls.
  - ACT fuses tanh(ue + wh) using per-partition bias = wh^T[:, b] column.
  - score = v . tanh via PE: lhsT = v chunk [128,1], rhs = tanh tile [128,512],
    accumulated over o-chunks into psum [1, 512].
  - softmax over S without max-subtraction (|score| <= sum|v_a| ~ 16, exp safe
    in fp32); exp on ACT with accum_out partial sums; reciprocal on DVE.
  - attn row round-trips through a DRAM scratch to get attn^T [128,16] per b.
  - context[b] = attn^T-cols @ enc natural-layout tiles [128 s, 512 h],
    accumulated over 16 s-chunks into psum [1, 512] per h-half.
"""

import sys

sys.path.insert(0, "/opt/trn_rl_repo")

import numpy as np

import concourse.bass as bass
import concourse.tile as tile
from concourse import bacc, mybir
from concourse import bass_utils

F32 = mybir.dt.float32
F32R = mybir.dt.float32r
BF16 = mybir.dt.bfloat16

N_CORES = 8
B = 32
S_FULL = 2048
H = 1024
B_L = B // N_CORES  # 4 batches per core

HC = H // 128  # 8 h-chunks
OC = H // 128  # 8 o-chunks


def build_program(reps: int = 1, mode: str = "f32r", s: int = S_FULL):
    """Build the per-core Bass program.

    reps > 1 wraps the whole body in a tc.For_i loop for wall-clock timing.
    mode: "f32r" (fp32 data, float32r matmuls), "bf16" (bf16 data/matmuls),
          "mix" (bf16 stationary weights + float32r moving operands),
          "f32"  (plain fp32 matmuls, 4x slower - baseline only).
    """
    assert s % 512 == 0 and (s // 512) >= 1
    R = B_L * s
    NBLK = R // 512  # r-blocks of 512 rows
    BPB = s // 512  # r-blocks per batch element

    # wt_dt: dtype of big stationary matmul operands (U_a^T, W_a^T tiles).
    # mv_dt: dtype of moving operands and trivial-column stationary operands.
    wt_dt, mv_dt = {"f32r": (F32R, F32R), "mix": (BF16, F32R),
                    "bf16": (BF16, BF16), "f32": (F32, F32)}[mode]
    in_dt = mv_dt

    def mmcast(ap):
        return ap

    nc = bacc.Bacc("TRN2", target_bir_lowering=False, debug=False,
                   num_devices=N_CORES)

    xT = nc.dram_tensor("xT", [H, R], in_dt, kind="ExternalInput").ap()
    enc = nc.dram_tensor("enc", [B_L, s, H], in_dt, kind="ExternalInput").ap()
    uaT = nc.dram_tensor("uaT", [H, H], wt_dt, kind="ExternalInput").ap()
    waT = nc.dram_tensor("waT", [H, H], wt_dt, kind="ExternalInput").ap()
    hidT = nc.dram_tensor("hidT", [H, B_L], in_dt, kind="ExternalInput").ap()
    vv = nc.dram_tensor("vv", [128, OC], in_dt, kind="ExternalInput").ap()
    ctx_out = nc.dram_tensor("ctx_out", [B_L, H], F32, kind="ExternalOutput").ap()
    attn_out = nc.dram_tensor("attn_out", [B_L, s], F32, kind="ExternalOutput").ap()

    with tile.TileContext(nc) as tc:
        with (
            tc.tile_pool(name="consts", bufs=1) as consts,
            tc.tile_pool(name="xt", bufs=2) as xt_p,
            tc.tile_pool(name="tanh", bufs=3) as tanh_p,
            tc.tile_pool(name="softmax", bufs=2) as sm_p,
            tc.tile_pool(name="encp", bufs=2) as enc_p,
            tc.tile_pool(name="attnT", bufs=2) as attnT_p,
            tc.tile_pool(name="small", bufs=4) as small_p,
            tc.tile_pool(name="uep", bufs=3, space="PSUM") as ue_ps_p,
            tc.tile_pool(name="scorep", bufs=2, space="PSUM") as score_ps_p,
            tc.tile_pool(name="ctxp", bufs=2, space="PSUM") as ctx_ps_p,
            tc.tile_pool(name="dram", bufs=2, space="DRAM") as dram_p,
        ):
            def body(_i=None):
                # ---- constants ----
                uaT_sb = consts.tile([128, HC, H], wt_dt, tag="uaT")
                nc.sync.dma_start(
                    uaT_sb[:], uaT.rearrange("(c p) o -> p c o", p=128))
                waT_sb = consts.tile([128, HC, H], wt_dt, tag="waT")
                nc.sync.dma_start(
                    waT_sb[:], waT.rearrange("(c p) o -> p c o", p=128))
                hidT_sb = consts.tile([128, HC, B_L], in_dt, tag="hidT")
                nc.sync.dma_start(
                    hidT_sb[:], hidT.rearrange("(c p) b -> p c b", p=128))
                v_sb = consts.tile([128, OC], in_dt, tag="v")
                nc.sync.dma_start(v_sb[:], vv[:])
                whT_sb = consts.tile([128, OC, B_L], F32, tag="whT")

                # ---- wh^T = W_a @ hidden^T  -> [o, b] per o-chunk ----
                for oc in range(OC):
                    whT_ps = ue_ps_p.tile([128, B_L], F32, tag="ue")
                    for hc in range(HC):
                        nc.tensor.matmul(
                            whT_ps[:],
                            mmcast(waT_sb[:, hc, oc * 128:(oc + 1) * 128]),
                            mmcast(hidT_sb[:, hc, :]),
                            start=(hc == 0), stop=(hc == HC - 1),
                        )
                    nc.scalar.copy(whT_sb[:, oc, :], whT_ps[:])

                # state carried per batch element
                exp_sb = {}
                sums_sb = {}
                inv_sb = {}

                def emit_block(rb):
                    b = rb // BPB
                    xt_t = xt_p.tile([128, HC, 512], in_dt, tag="xt")
                    nc.sync.dma_start(
                        xt_t[:],
                        xT.rearrange("(c p) r -> p c r", p=128)[
                            :, :, rb * 512:(rb + 1) * 512],
                    )
                    score_ps = score_ps_p.tile([1, 512], F32, tag="score")
                    pend = []  # delayed v-dot matmuls to keep PE dense
                    for oc in range(OC):
                        ue_ps = ue_ps_p.tile([128, 512], F32, tag="ue")
                        for hc in range(HC):
                            nc.tensor.matmul(
                                ue_ps[:],
                                mmcast(uaT_sb[:, hc, oc * 128:(oc + 1) * 128]),
                                mmcast(xt_t[:, hc, :]),
                                start=(hc == 0), stop=(hc == HC - 1),
                            )
                        tanh_t = tanh_p.tile([128, 512], in_dt, tag="tanh")
                        nc.scalar.activation(
                            tanh_t[:], ue_ps[:],
                            mybir.ActivationFunctionType.Tanh,
                            bias=whT_sb[:, oc, b:b + 1],
                        )
                        if pend:
                            pend.pop()()
                        pend.append(lambda oc=oc, t=tanh_t: nc.tensor.matmul(
                            score_ps[:],
                            mmcast(v_sb[:, oc:oc + 1]),
                            mmcast(t[:]),
                            start=(oc == 0), stop=(oc == OC - 1),
                        ))
                    pend.pop()()

                    # exp of this block's scores, with partial sum
                    j = rb % BPB
                    if j == 0:
                        exp_sb[b] = sm_p.tile([1, s], F32, tag="exp", name=f"exp{b}")
                        sums_sb[b] = small_p.tile([1, BPB], F32, tag="sums", name=f"sums{b}")
                    nc.scalar.activation(
                        exp_sb[b][:, j * 512:(j + 1) * 512], score_ps[:],
                        mybir.ActivationFunctionType.Exp,
                        accum_out=sums_sb[b][:, j:j + 1],
                    )

                def emit_softmax(b):
                    tot = small_p.tile([1, 1], F32, tag="tot")
                    nc.vector.tensor_reduce(
                        tot[:], sums_sb[b][:], axis=mybir.AxisListType.X,
                        op=mybir.AluOpType.add)
                    inv_sb[b] = small_p.tile([1, 1], F32, tag="inv", name=f"inv{b}")
                    nc.vector.reciprocal(inv_sb[b][:], tot[:])
                    attn_row = exp_sb[b]  # normalize in place
                    nc.vector.tensor_scalar_mul(
                        attn_row[:], exp_sb[b][:], inv_sb[b][:])
                    nc.sync.dma_start(attn_out[b:b + 1, :], attn_row[:])
                    # matmul-typed copy of the row for the transpose round-trip
                    attn_row_mm = sm_p.tile([1, s], in_dt, tag="attnmm",
                                            name=f"attnmm{b}")
                    nc.vector.tensor_copy(attn_row_mm[:], attn_row[:])
                    scr = dram_p.tile([1, s], in_dt, tag="scr")
                    nc.sync.dma_start(scr[:], attn_row_mm[:])
                    attnT = attnT_p.tile([128, s // 128], in_dt, tag="attnT")
                    nc.sync.dma_start(
                        attnT[:], scr[0].rearrange("(c p) -> p c", p=128))
                    return attnT

                def emit_context(b, attnT):
                    nsc = s // 128  # s-chunks
                    ngrp = max(1, nsc // 8)  # DMA groups of 8 s-chunks
                    gsz = nsc // ngrp
                    ctx_sb = small_p.tile([1, H], F32, tag="ctx")
                    for half in range(2):
                        ctx_ps = ctx_ps_p.tile([1, 512], F32, tag="ctx")
                        for g in range(ngrp):
                            enc_t = enc_p.tile([128, gsz, 512], in_dt, tag="enc")
                            nc.sync.dma_start(
                                enc_t[:],
                                enc[b].rearrange("(c p) (n h) -> p c n h",
                                                 p=128, n=2)[
                                    :, g * gsz:(g + 1) * gsz, half, :],
                            )
                            for k in range(gsz):
                                sc = g * gsz + k
                                nc.tensor.matmul(
                                    ctx_ps[:],
                                    mmcast(attnT[:, sc:sc + 1]),
                                    mmcast(enc_t[:, k, :]),
                                    start=(sc == 0), stop=(sc == nsc - 1),
                                )
                        nc.scalar.copy(
                            ctx_sb[:, half * 512:(half + 1) * 512], ctx_ps[:])
                    nc.sync.dma_start(ctx_out[b:b + 1, :], ctx_sb[:])

                attnT_ready = {}
                for rb in range(NBLK):
                    emit_block(rb)
                    b = rb // BPB
                    if rb % BPB == BPB - 1:
                        attnT_ready[b] = emit_softmax(b)
                    # emit context for previous b one block into the next b,
                    # so PE has ue work while softmax b completes
                    if rb % BPB == min(1, BPB - 1) and b > 0:
                        emit_context(b - 1, attnT_ready[b - 1])
                emit_context(B_L - 1, attnT_ready[B_L - 1])

            if reps == 1:
                body()
            else:
                with tc.For_i(0, reps, 1) as i:
                    body(i)

    nc.compile()
    return nc


def shard_inputs(hidden, encoder_outputs, W_a, U_a, v_a, mode="f32r",
                 s=S_FULL):
    """Build the 8 per-core input maps (numpy only; layout prep)."""
    import ml_dtypes
    np_mv = ml_dtypes.bfloat16 if mode == "bf16" else np.float32
    np_wt = ml_dtypes.bfloat16 if mode in ("bf16", "mix") else np.float32

    def cast(x, dt=None):
        return np.ascontiguousarray(x, dtype=dt or np_mv)

    uaT = cast(np.asarray(U_a, np.float32).T, np_wt)
    waT = cast(np.asarray(W_a, np.float32).T, np_wt)
    vv = cast(np.asarray(v_a, np.float32).reshape(OC, 128).T)
    in_maps = []
    for c in range(N_CORES):
        bl = slice(c * B_L, (c + 1) * B_L)
        enc_l = np.asarray(encoder_outputs[bl], np.float32)
        hid_l = np.asarray(hidden[bl], np.float32)
        in_maps.append({
            "xT": cast(enc_l.reshape(B_L * s, H).T),
            "enc": cast(enc_l),
            "uaT": uaT,
            "waT": waT,
            "hidT": cast(hid_l.T),
            "vv": vv,
        })
    return in_maps


_prog_cache = {}


def _get_prog(reps=1, mode="f32r", s=S_FULL):
    key = (reps, mode, s)
    if key not in _prog_cache:
        _prog_cache[key] = build_program(reps, mode, s)
    return _prog_cache[key]


def run(hidden, encoder_outputs, W_a, U_a, v_a, mode="f32r", s=S_FULL,
        reps=1):
    nc = _get_prog(reps, mode, s)
    in_maps = shard_inputs(hidden, encoder_outputs, W_a, U_a, v_a, mode, s)
    res = bass_utils.run_bass_kernel_spmd(nc, in_maps,
                                          core_ids=list(range(N_CORES)))
    context = np.concatenate([res.results[c]["ctx_out"] for c in range(N_CORES)])
    attn = np.concatenate([res.results[c]["attn_out"] for c in range(N_CORES)])
    return context, attn


def kernel(hidden, encoder_outputs, W_a, U_a, v_a):
    return run(np.asarray(hidden), np.asarray(encoder_outputs),
               np.asarray(W_a), np.asarray(U_a), np.asarray(v_a))
